# revision 9
# baseline (speedup 1.0000x reference)
"""Trainium2 Bass kernel for nn_DeTokenizer (ragged EMA scan + gather + residual).

Contract: kernel(**inputs) takes FULL unsharded inputs (np arrays, keyed as in
setup_inputs()) and returns (output, new_state) matching reference().

Strategy (pure data parallel, one batch row per NeuronCore):
  - Host precomputes (fp64) per-chunk scan coefficient matrices from
    prob/token_mask/counts only. All O(D)-sized math runs on device.
  - EMA over M=2048 chunks = 16 blocked matmuls LT_c @ x_c (fp32) plus an
    inter-chunk carry h_in computed in ONE accumulated matmul group
    (h_in[c] = sum_j V_j @ x_j + g~ (x) state), then K=16 correction matmuls.
  - Chunk-state table kept in SBUF as bf16 hi+lo planes (hi+lo reconstruction
    is exact to ~4e-6 rel).
  - Gather to L=4096 positions: indices are monotone, so each 128-position
    tile reads <=2 aligned 128-row chunks. One-hot selection matrices are
    built ON DEVICE (K=1 broadcast matmul + is_equal against host iota
    columns); table chunk picked via dynamic (register) free-dim offsets.
  - Residual add on DVE, stream out.
"""

import os
import numpy as np

B, L, M, D = 8, 4096, 2048, 512
C = 128           # chunk size
NCH = M // C      # 16 chunks
NPT = L // C      # 32 position tiles
N_CORES = 8

_CACHED = {}
LAST_EXEC_TIME_NS = None


# ---------------------------------------------------------------------------
# compile-environment patches: this walrus build accepts at most one sync wait
# per instruction (two for EventSemaphore) and cannot encode SeqAssert.
# ---------------------------------------------------------------------------
def _apply_patches():
    import json
    import concourse.bass as _bass
    import concourse.mybir as mybir
    from concourse import tile as _tile
    from concourse.bass import _bass_rust

    if getattr(_bass.Bass, "_detok_patched", False):
        return

    def _drain_and_barrier(self, tick_clock, wait_clock):
        nc = self.nc
        drain_inst = nc.sync.drain()
        wait_clock.add_sem_waits(
            drain_inst.ins, _bass_rust.ScopedClock({None: tick_clock.global_clock})
        )
        si = drain_inst.ins.sync_info
        waits = list(si.on_wait) if si is not None and si.on_wait else []
        if len(waits) > 1:
            si.on_wait = [waits[0]]
            for w in waits[1:]:
                nop = nc.sync.nop()
                nsi = nop.ins.sync_info
                if nsi is None:
                    nop.ins.sync_info = mybir.SyncInfo(on_wait=[w], on_update=[])
                else:
                    nsi.on_wait = [w]
        nc.all_engine_barrier()
        assert self.sems is not None
        popped = nc._tile_sem_poison_stack.pop()
        assert popped is self._sem_poison
        nc.clear_and_free_semaphores(list(self.sems.allocated().values()))
        nc.all_engine_barrier()

    _tile.TileContext._drain_and_barrier = _drain_and_barrier

    _orig_to_json = _bass.Bass.to_json_bytes
    ctr = [0]

    def _split_waits_json(data: bytes) -> bytes:
        d = json.loads(data)
        changed = False
        for fn in d.get("functions", []):
            for blk in fn.get("blocks", []):
                insts = blk.get("instructions")
                if not insts:
                    continue
                out = []
                for inst in insts:
                    if inst.get("opcode") == "ISA" and inst.get("op_name") == "SeqAssert":
                        inst = {
                            "debug": inst.get("debug", 0),
                            "engine": inst.get("engine"),
                            "ins": [], "outs": [],
                            "name": inst["name"],
                            "opcode": "NoOp",
                            "sync_info": inst.get("sync_info")
                            or {"on_update": [], "on_wait": []},
                        }
                        changed = True
                    si = inst.get("sync_info") or {}
                    ow = si.get("on_wait") or []
                    limit = 2 if inst.get("opcode") == "EventSemaphore" else 1
                    if len(ow) > limit and inst.get("engine"):
                        extra, keep = ow[:-limit], ow[-limit:]
                        for w in extra:
                            ctr[0] += 1
                            out.append({
                                "debug": inst.get("debug", 0),
                                "engine": inst["engine"],
                                "ins": [], "outs": [],
                                "name": f"I-wsplit-{ctr[0]}",
                                "opcode": "NoOp",
                                "sync_info": {"on_update": [], "on_wait": [w]},
                            })
                        si["on_wait"] = keep
                        changed = True
                    out.append(inst)
                blk["instructions"] = out
        if not changed:
            return data
        return json.dumps(d).encode()

    def _to_json_bytes(self, *a, **k):
        return _split_waits_json(_orig_to_json(self, *a, **k))

    _bass.Bass.to_json_bytes = _to_json_bytes
    _bass.Bass._detok_patched = True


# ---------------------------------------------------------------------------
# device program (static SPMD; all data-dependence flows through tensors)
# ---------------------------------------------------------------------------
def _build_program(rep_ab=1, rep_c=1):
    _apply_patches()
    import concourse.bass as bass
    import concourse.bacc as bacc
    import concourse.mybir as mybir
    from concourse.tile import TileContext

    F32 = mybir.dt.float32
    F16 = mybir.dt.float16
    BF16 = mybir.dt.bfloat16
    I32 = mybir.dt.int32
    EQ = mybir.AluOpType.is_equal

    nc = bacc.Bacc("TRN2", target_bir_lowering=False, debug=False,
                   enable_asserts=False, num_devices=N_CORES)

    X    = nc.declare_dram_parameter("X",    [M, D],        F32, isOutput=False)
    LTP  = nc.declare_dram_parameter("LTP",  [C, M],        F32, isOutput=False)
    VJ   = nc.declare_dram_parameter("VJ",   [C, NCH * NCH],F32, isOutput=False)
    PT   = nc.declare_dram_parameter("PT",   [C, M],        F32, isOutput=False)
    GS   = nc.declare_dram_parameter("GS",   [1, NCH],      F32, isOutput=False)
    ST   = nc.declare_dram_parameter("ST",   [1, D],        F32, isOutput=False)
    RES  = nc.declare_dram_parameter("RES",  [L, D],        F32, isOutput=False)
    IDXF = nc.declare_dram_parameter("IDXF", [1, L],        F16, isOutput=False)
    CMPA = nc.declare_dram_parameter("CMPA", [C, NPT],      F32, isOutput=False)
    CMPB = nc.declare_dram_parameter("CMPB", [C, NPT],      F32, isOutput=False)
    OFFS = nc.declare_dram_parameter("OFFS", [1, NPT],      I32, isOutput=False)
    SNS  = nc.declare_dram_parameter("SNS",  [C, 1],        BF16, isOutput=False)
    OFFN = nc.declare_dram_parameter("OFFN", [1, 1],        I32, isOutput=False)
    ONEV = nc.declare_dram_parameter("ONEV", [1, C],        F16, isOutput=False)

    OUT = nc.declare_dram_parameter("OUT", [L, D], F32, isOutput=True)
    NST = nc.declare_dram_parameter("NST", [1, D], F32, isOutput=True)

    TPAD = (NCH + 1) * D  # table free size incl. zero pad chunk

    with TileContext(nc) as tc:
        with (
            tc.tile_pool(name="big", bufs=1) as big,
            tc.tile_pool(name="resp", bufs=6) as resp,
            tc.tile_pool(name="outp", bufs=6) as outp,
            tc.tile_pool(name="sp", bufs=4) as sp,
            tc.tile_pool(name="ps", bufs=1, space="PSUM") as ps,
            tc.tile_pool(name="psc", bufs=4, space="PSUM") as psc,
            tc.tile_pool(name="psb", bufs=2, space="PSUM") as psb,
            tc.tile_pool(name="psg", bufs=2, space="PSUM") as psg,
        ):
            # --- persistent SBUF loads ---
            xc   = big.tile([C, NCH * D], F32, tag="xc")
            for c in range(NCH):
                nc.sync.dma_start(out=xc[:, c * D:(c + 1) * D],
                                  in_=X[c * C:(c + 1) * C, :])
            ltp  = big.tile([C, M], F32, tag="ltp");  nc.sync.dma_start(out=ltp[:], in_=LTP[:])
            vj   = big.tile([C, NCH * NCH], F32, tag="vj"); nc.sync.dma_start(out=vj[:], in_=VJ[:])
            pt   = big.tile([C, M], F32, tag="pt");   nc.sync.dma_start(out=pt[:], in_=PT[:])
            gs   = big.tile([1, NCH], F32, tag="gs"); nc.sync.dma_start(out=gs[:], in_=GS[:])
            st   = big.tile([1, D], F32, tag="st");   nc.sync.dma_start(out=st[:], in_=ST[:])
            idxf = big.tile([1, L], F16, tag="idxf"); nc.sync.dma_start(out=idxf[:], in_=IDXF[:])
            cmpa = big.tile([C, NPT], F32, tag="cmpa"); nc.sync.dma_start(out=cmpa[:], in_=CMPA[:])
            cmpb = big.tile([C, NPT], F32, tag="cmpb"); nc.sync.dma_start(out=cmpb[:], in_=CMPB[:])
            offs = big.tile([1, NPT], I32, tag="offs"); nc.sync.dma_start(out=offs[:], in_=OFFS[:])
            sns  = big.tile([C, 1], BF16, tag="sns");  nc.sync.dma_start(out=sns[:], in_=SNS[:])
            offn = big.tile([1, 1], I32, tag="offn");  nc.sync.dma_start(out=offn[:], in_=OFFN[:])
            onev = big.tile([1, C], F16, tag="onev");  nc.sync.dma_start(out=onev[:], in_=ONEV[:])

            tbl_hi = big.tile([C, TPAD], BF16, tag="tbl_hi")
            tbl_lo = big.tile([C, TPAD], BF16, tag="tbl_lo")
            nc.vector.memset(tbl_hi[:, NCH * D:], 0.0)
            nc.vector.memset(tbl_lo[:, NCH * D:], 0.0)

            hin = big.tile([C, D], F32, tag="hin")   # h_in replicated at 0/32/64/96

            offn_v = nc.tensor.value_load(offn[0:1, 0:1], min_val=0,
                                          max_val=(NCH - 1) * D)

            for _rep in range(rep_ab):
              # --- phase A: h_in table [NCH, D] in one PSUM accumulation ---
              ph4 = psc.tile([NCH, D], F32, tag="pc")
              for j in range(NCH):
                  nc.tensor.matmul(ph4[:], vj[:, j * NCH:(j + 1) * NCH],
                                   xc[:, j * D:(j + 1) * D],
                                   start=(j == 0), stop=False, skip_group_check=True)
              nc.tensor.matmul(ph4[:], gs[:], st[:], start=False, stop=True,
                               skip_group_check=True)
              nc.scalar.copy(hin[0:NCH, :], ph4[:])

              # --- phase B: chunk-state table ---
              for c in range(NCH):
                  pc = psc.tile([C, D], F32, tag="pc")
                  nc.tensor.matmul(pc[:], ltp[:, c * C:(c + 1) * C],
                                   xc[:, c * D:(c + 1) * D],
                                   start=True, stop=False, skip_group_check=True)
                  nc.tensor.matmul(pc[:], pt[0:NCH, c * C:(c + 1) * C],
                                   hin[0:NCH, :],
                                   start=False, stop=True,
                                   skip_group_check=True)
                  hi_sl = tbl_hi[:, c * D:(c + 1) * D]
                  nc.scalar.copy(hi_sl, pc[:])
                  nc.vector.tensor_sub(tbl_lo[:, c * D:(c + 1) * D], pc[:], hi_sl)

              # --- new_state row ---
              pns = psc.tile([1, D], F32, tag="pc")
              nc.tensor.matmul(pns[:], sns[:], tbl_hi[:, bass.ds(offn_v, D)],
                               start=True, stop=False, skip_group_check=True)
              nc.tensor.matmul(pns[:], sns[:], tbl_lo[:, bass.ds(offn_v, D)],
                               start=False, stop=True, skip_group_check=True)
              nso = big.tile([1, D], F32, tag="nso")
              nc.scalar.copy(nso[:], pns[:])
              nc.sync.dma_start(out=NST[:], in_=nso[:])

            # --- phase C: gather + residual add, 8 groups x 4 position tiles ---
            PE = mybir.EngineType.PE
            offvals_all = []
            for g in range(NPT // 8):
                _, offvals = nc.values_load_multi_w_load_instructions(
                    offs[0:1, g * 8:(g + 1) * 8], engines=(PE,),
                    min_val=0, max_val=(NCH - 1) * D)
                offvals_all.extend(offvals)
            for _rep in range(rep_c):
             for g in range(NPT // 4):
                # one fp16 K=1 matmul broadcasts 4 tiles of indices
                pb = psb.tile([C, 4 * C], F32, tag="pb")
                nc.tensor.matmul(pb[:], onev[:], idxf[0:1, g * 4 * C:(g + 1) * 4 * C],
                                 start=True, stop=True, skip_group_check=True)
                for k in range(4):
                    i = g * 4 + k
                    ob = offvals_all[i]
                    rt = resp.tile([C, D], F32, tag="rt")
                    nc.sync.dma_start(out=rt[:], in_=RES[i * C:(i + 1) * C, :])

                    sa = sp.tile([C, C], BF16, tag="sa")
                    nc.vector.tensor_scalar(out=sa[:], in0=pb[:, k * C:(k + 1) * C],
                                            scalar1=cmpa[:, i:i + 1], scalar2=None,
                                            op0=EQ)
                    sb_ = sp.tile([C, C], BF16, tag="sb")
                    nc.vector.tensor_scalar(out=sb_[:], in0=pb[:, k * C:(k + 1) * C],
                                            scalar1=cmpb[:, i:i + 1], scalar2=None,
                                            op0=EQ)

                    pg = psg.tile([C, D], F32, tag="pg")
                    nc.tensor.matmul(pg[:], sa[:], tbl_hi[:, bass.ds(ob, D)],
                                     start=True, stop=False, skip_group_check=True)
                    nc.tensor.matmul(pg[:], sa[:], tbl_lo[:, bass.ds(ob, D)],
                                     start=False, stop=False, skip_group_check=True)
                    ob2 = ob + D
                    nc.tensor.matmul(pg[:], sb_[:], tbl_hi[:, bass.ds(ob2, D)],
                                     start=False, stop=False, skip_group_check=True)
                    nc.tensor.matmul(pg[:], sb_[:], tbl_lo[:, bass.ds(ob2, D)],
                                     start=False, stop=True, skip_group_check=True)

                    ot = outp.tile([C, D], F32, tag="ot")
                    nc.vector.tensor_add(ot[:], pg[:], rt[:])
                    nc.sync.dma_start(out=OUT[i * C:(i + 1) * C, :], in_=ot[:])

    nc.compile()
    return nc


# ---------------------------------------------------------------------------
# host-side coefficient precompute (fp64; depends only on prob/mask/counts)
# ---------------------------------------------------------------------------
def _host_prep_row(prob_b, mask_b, count_b, state_b, hidden_b, residual_b):
    cnt = int(count_b)
    pr = np.asarray(prob_b, np.float64)
    mask = np.asarray(mask_b, bool)

    order = np.argsort(~mask, kind="stable")[:M]
    cp = pr[order]
    valid = np.arange(M) < cnt
    cp = np.where(valid, cp, 0.0)
    decay = np.clip(1.0 - cp, 0.0, 1.0)          # (M,)
    onem = 1.0 - decay

    dch = decay.reshape(NCH, C)
    onch = onem.reshape(NCH, C)
    vch = valid.reshape(NCH, C)

    # R_c[s,t] = prod_{u=s+1..t} d[u]  (t>=s), per chunk
    LTP = np.zeros((C, M), np.float32)           # lhsT: [s, c*C + t]
    PTm = np.zeros((C, M), np.float32)           # row-tiled K=16 lhsT, replicas at 32*r
    lastcol = np.zeros((NCH, C), np.float64)     # r_c coefficients (unzeroed)
    pf = np.zeros(NCH, np.float64)
    for c in range(NCH):
        d = dch[c]
        R = np.zeros((C, C), np.float64)
        col = np.zeros(C, np.float64)
        for t in range(C):
            col = col * d[t]
            col[t] = 1.0
            R[:, t] = col
        P = np.cumprod(d)                        # inclusive cumprod
        Lc = onch[c][:, None] * R                # [s, t]
        lastcol[c] = Lc[:, C - 1]
        pf[c] = P[C - 1]
        # validity zeroing of output rows (columns t of lhsT layout)
        Lc = Lc * vch[c][None, :]
        Pz = P * vch[c]
        LTP[:, c * C:(c + 1) * C] = Lc.astype(np.float32)
        PTm[c, c * C:(c + 1) * C] = Pz.astype(np.float32)

    # G[c,j] = prod_{j<k<c} pf_k ; gtilde[c] = prod_{k<c} pf_k
    VJm = np.zeros((C, NCH * NCH), np.float32)   # V_j at cols j*NCH..: [s, c]
    gt = np.zeros(NCH, np.float64)
    for c in range(NCH):
        gt[c] = np.prod(pf[:c]) if c > 0 else 1.0
    for j in range(NCH):
        for c in range(j + 1, NCH):
            G = np.prod(pf[j + 1:c]) if c > j + 1 else 1.0
            VJm[:, j * NCH + c] = (G * lastcol[j]).astype(np.float32)
    GSm = gt.astype(np.float32).reshape(1, NCH)

    # gather metadata
    chunk_idx = np.cumsum(mask.astype(np.int64)) - 1          # (L,), -1 allowed
    idxf = chunk_idx.astype(np.float16).reshape(1, L)
    iota = np.arange(C, dtype=np.float32)
    cmpa = np.zeros((C, NPT), np.float32)
    cmpb = np.zeros((C, NPT), np.float32)
    offA = np.zeros((1, NPT), np.int32)
    ci = chunk_idx.reshape(NPT, C)
    for i in range(NPT):
        ids = ci[i]
        pos = ids[ids >= 0]
        cA = int(pos.min() // C) if pos.size else 0
        offA[0, i] = cA * D
        cmpa[:, i] = iota + cA * C
        cmpb[:, i] = iota + (cA + 1) * C

    # new_state one-hot
    import ml_dtypes
    sns = np.zeros((C, 1), ml_dtypes.bfloat16)
    offn = np.zeros((1, 1), np.int32)
    if cnt > 0:
        sns[(cnt - 1) % C, 0] = 1.0
        offn[0, 0] = ((cnt - 1) // C) * D

    return {
        "X": np.ascontiguousarray(hidden_b, np.float32),
        "LTP": LTP,
        "VJ": VJm,
        "PT": PTm,
        "GS": GSm,
        "ST": np.asarray(state_b, np.float32).reshape(1, D),
        "RES": np.ascontiguousarray(residual_b, np.float32),
        "IDXF": idxf,
        "CMPA": cmpa,
        "CMPB": cmpb,
        "OFFS": offA,
        "SNS": sns,
        "OFFN": offn,
        "ONEV": np.ones((1, C), np.float16),
    }


def kernel(hidden_states, residual, prob, state, token_mask, counts):
    global LAST_EXEC_TIME_NS
    hidden_states = np.asarray(hidden_states, np.float32)
    residual = np.asarray(residual, np.float32)
    prob = np.asarray(prob, np.float32)
    state = np.asarray(state, np.float32)
    token_mask = np.asarray(token_mask, bool)
    counts = np.asarray(counts, np.int32)

    from concourse.bass_utils import run_bass_kernel_spmd

    if "nc" not in _CACHED:
        _CACHED["nc"] = _build_program()
    nc = _CACHED["nc"]

    in_maps = [
        _host_prep_row(prob[b], token_mask[b], counts[b], state[b],
                       hidden_states[b], residual[b])
        for b in range(B)
    ]

    trace = bool(int(os.environ.get("KERNEL_TRACE", "0")))
    res = run_bass_kernel_spmd(nc, in_maps, list(range(N_CORES)), trace=trace)
    LAST_EXEC_TIME_NS = getattr(res, "exec_time_ns", None)

    output = np.stack([res.results[b]["OUT"] for b in range(B)], axis=0)
    new_state = np.stack(
        [res.results[b]["NST"].reshape(D) for b in range(B)], axis=0
    )
    # count==0 fallback: device row is zeros; reference keeps incoming state
    for b in range(B):
        if int(counts[b]) == 0:
            new_state[b] = state[b]
    return output.astype(np.float32), new_state.astype(np.float32)
